# revision 11
# baseline (speedup 1.0000x reference)
"""TRN2 Bass kernel for nn_CustomLinear_66005057405513.

Computes y = FFT_4096(w * x)[:, :3072] for x: [4096, 4096] complex64
(given as interleaved float pairs) and w: [4096] complex64 twiddles.

Strategy: data-parallel over 8 NeuronCores (512 batch rows each). On each
core, a two-stage radix-64 FFT with all twiddles folded into precomputed
matrices:

  n = 64*o + i, k = p + 64*q, q < 48:
    stage 1 (per i):  A[b, i, p] = sum_o C1[i][o, p] * x[b, 64o+i]
                      C1[i][o, p] = W64^(op) * w[64o+i]
    stage 2 (per p):  y[b, p+64q] = sum_i C2[p][i, q] * A[b, i, p]
                      C2[p][i, q] = W4096^(ip) * W64^(iq)

Complex values ride as interleaved (re, im) float pairs; each complex
matmul is one real matmul with the pair-encoded matrix (contraction
K = 128 = full PE partition dim), fp16 compute with fp32 PSUM accum.

The kernel is PE-sequencer-bound: every matmul/transpose costs ~100ns
of issue (Ldweights + dispatch) regardless of size, so the design
minimizes PE instruction count (576 pairs/iteration):

- x is pre-transposed on the host to [oc, (slab, i, b)]: stage 1 needs
  no transposes (256 pairs, N=128 forced by the per-i tables).
- The structurally-required corner-turn of A runs as 256 PE transposes.
- Stage 2 is 64 pairs: one N=512 matmul per p (stationary = the
  constant C2[p] table, moving = all 512 batch columns of transposed A).
- Stage-2 p-range g follows transpose group g of the last slab, so the
  tail interleaves and PSUM never serializes the pipeline.
- Output is y^T [qc, (p, s, b)]; the host untangles it for free.
"""

import numpy as np

import concourse.bass as bass
import concourse.mybir as mybir
from concourse import bacc
from concourse.tile import TileContext
from concourse.masks import make_identity
from concourse.bass_utils import run_bass_kernel_spmd

O = I = 64
N_FFT = O * I          # 4096
Q = 48                 # q < 48  <=>  k < 3072
QC = 2 * Q             # 96
B_TOTAL = 4096
N_CORES = 8
B_LOCAL = B_TOTAL // N_CORES  # 512
SLABS = B_LOCAL // 128        # 4
YCOLS = O * B_LOCAL           # 32768


def _make_tables(w_complex):
    oo = np.arange(O)
    W64 = np.exp(-2j * np.pi * np.outer(oo, oo) / O)
    WN = np.exp(-2j * np.pi * np.outer(np.arange(I), oo) / N_FFT)

    def pairmat(C):
        K, M = C.shape
        G = np.empty((2 * K, 2 * M), np.float64)
        G[0::2, 0::2] = C.real
        G[1::2, 0::2] = -C.imag
        G[0::2, 1::2] = C.imag
        G[1::2, 1::2] = C.real
        return G

    g1 = np.empty((128, I, 128), np.float64)
    for i in range(I):
        C1 = W64 * w_complex[64 * oo + i][:, None]
        g1[:, i, :] = pairmat(C1)
    g2 = np.empty((128, O, QC), np.float64)
    for p in range(O):
        C2 = WN[:, p][:, None] * W64[:, :Q]
        g2[:, p, :] = pairmat(C2)
    return g1, g2


def _build_nc(reps=1, unroll=False):
    f32 = mybir.dt.float32
    f16 = mybir.dt.float16

    nc = bacc.Bacc(None, target_bir_lowering=False, debug=False)
    # x host layout: [oc, (slab, i, b)] -- already transposed for stage 1
    x = nc.declare_dram_parameter("x", [128, SLABS * 8192], f16, isOutput=False)
    w1 = nc.declare_dram_parameter("w1", [128, I * 128], f16, isOutput=False)
    w2 = nc.declare_dram_parameter("w2", [128, O * QC], f16, isOutput=False)
    # y device layout: [qc, (p, s, b)] -- host untangles to [b, k]
    y = nc.declare_dram_parameter("y", [QC, YCOLS], f16, isOutput=True)

    yc = [0]

    def copy_y(out_ap, in_ap):
        # ~25 of every 64 y-evacs on ACT, rest on DVE (measured balance)
        k = yc[0] % 8
        yc[0] += 1
        if k in (0, 3, 5):
            nc.scalar.copy(out_ap, in_ap)
        else:
            nc.vector.tensor_copy(out_ap, in_ap)

    with TileContext(nc) as tc:
        with (
            tc.tile_pool(name="const", bufs=1) as cpool,
            tc.tile_pool(name="xp", bufs=2) as xpool,
            tc.tile_pool(name="ap", bufs=2) as apool,
            tc.tile_pool(name="at", bufs=1) as atpool,
            tc.tile_pool(name="yp", bufs=2) as ypool,
            tc.tile_pool(name="pm1", bufs=2, space="PSUM") as pm1,
            tc.tile_pool(name="pt2", bufs=2, space="PSUM") as pt2,
            tc.tile_pool(name="pm2", bufs=2, space="PSUM") as pm2,
        ):
            ident = cpool.tile([128, 128], f16, name="ident")
            make_identity(nc, ident[:])
            w1s = cpool.tile([128, I * 128], f16, name="w1s")
            nc.scalar.dma_start(out=w1s[:], in_=w1[:])
            w2s = cpool.tile([128, O * QC], f16, name="w2s")
            nc.scalar.dma_start(out=w2s[:], in_=w2[:])
            w1v = w1s[:].rearrange("k (i n) -> k i n", i=I)
            w2v = w2s[:].rearrange("k (p n) -> k p n", p=O)

            def job(_iv=None):
                # At: [ic, (s, p, b)] -- transposed A, written contiguously
                At = atpool.tile([128, SLABS * 8192], f16, name="At")
                atv = At[:].rearrange("k (s p b) -> k p s b", s=SLABS, p=O)
                live = {}

                def s1_group(g):
                    xv, av = live["xv"], live["av"]
                    m1 = pm1.tile([128, 1024], f32, name="m1")
                    for j in range(8):
                        i = g * 8 + j
                        nc.tensor.matmul(
                            m1[:, j * 128:(j + 1) * 128],
                            lhsT=xv[:, i, :], rhs=w1v[:, i, :],
                            start=True, stop=True)
                    nc.scalar.copy(
                        av[:, g * 8:(g + 1) * 8, :, :],
                        m1[:].rearrange("b (i p c) -> b i p c", i=8, c=2))

                def tr_group(s, g):
                    Ab = live["Ab%d" % s]
                    t2p = pt2.tile([128, 1024], f16, name="t2p")
                    for j in range(8):
                        p = g * 8 + j
                        nc.tensor.transpose(
                            t2p[:, j * 128:(j + 1) * 128],
                            Ab[:, p * 128:(p + 1) * 128], ident[:])
                    base = s * 8192 + g * 1024
                    nc.vector.tensor_copy(At[:, base:base + 1024], t2p[:])

                def s2_one(p):
                    yst = live["yst"]
                    m2 = pm2.tile([QC, 512], f32, name="m2")
                    nc.tensor.matmul(
                        m2[:], lhsT=w2v[:, p, :], rhs=atv[:, p, :, :],
                        start=True, stop=True)
                    copy_y(yst[:, (p % 8) * 512:(p % 8 + 1) * 512], m2[:])

                for s in range(SLABS):
                    xs = xpool.tile([128, 8192], f16, name="xs")
                    for hh in range(2):
                        nc.sync.dma_start(
                            out=xs[:, hh * 4096:(hh + 1) * 4096],
                            in_=x[:, s * 8192 + hh * 4096:
                                  s * 8192 + (hh + 1) * 4096])
                    live["xv"] = xs[:].rearrange("k (i b) -> k i b", i=I)
                    Ab = apool.tile([128, 8192], f16, name="Ab")
                    live["Ab%d" % s] = Ab
                    live["av"] = Ab[:].rearrange("b (p i c) -> b i p c", p=O, c=2)
                    for g in range(8):
                        s1_group(g)
                        if s > 0:
                            tr_group(s - 1, g)
                # tail: tr(3, g) immediately followed by s2 of that p-range
                for g in range(8):
                    tr_group(SLABS - 1, g)
                    live["yst"] = ypool.tile([QC, 4096], f16, name="yst")
                    for p in range(8 * g, 8 * g + 8):
                        s2_one(p)
                    nc.scalar.dma_start(
                        out=y[:, g * 4096:(g + 1) * 4096], in_=live["yst"][:])

            if reps > 1 and unroll:
                for _ in range(reps):
                    job()
            elif reps > 1:
                with tc.For_i(0, reps, 1) as _i:
                    job(_i)
            else:
                job()

    nc.compile()
    return nc


_NC_CACHE = {}


def _get_nc():
    if "nc" not in _NC_CACHE:
        _NC_CACHE["nc"] = _build_nc()
    return _NC_CACHE["nc"]


def _host_inputs(x_real, weights_real):
    wr = np.asarray(weights_real, dtype=np.float64)
    wc = wr[0::2] + 1j * wr[1::2]
    g1, g2 = _make_tables(wc)
    w1 = np.ascontiguousarray(g1.reshape(128, -1)).astype(np.float16)
    w2 = np.ascontiguousarray(g2.reshape(128, -1)).astype(np.float16)
    x = np.asarray(x_real)
    B = x.shape[0]
    bl = B // N_CORES
    xh = x.reshape(B, O, I, 2).astype(np.float16)  # [b, o, i, c]
    maps = []
    for c in range(N_CORES):
        xc = xh[c * bl:(c + 1) * bl].reshape(SLABS, 128, O, I, 2)
        # [bt, b, o, i, c] -> [o, c, bt, i, b] = [oc, (slab, i, b)]
        xf = np.ascontiguousarray(xc.transpose(2, 4, 0, 3, 1)).reshape(128, -1)
        maps.append({"x": xf, "w1": w1, "w2": w2})
    return maps


def kernel(x_real, weights_real):
    nc = _get_nc()
    in_maps = _host_inputs(x_real, weights_real)
    res = run_bass_kernel_spmd(nc, in_maps, list(range(N_CORES)))
    outs = []
    for c in range(N_CORES):
        v = np.asarray(res.results[c]["y"], dtype=np.float32)
        # v[2q+c, (p, s, b)] -> Y[s*128+b, q*64+p]
        v = v.reshape(Q, 2, O, SLABS, 128).transpose(3, 4, 0, 2, 1)
        v = np.ascontiguousarray(v).reshape(B_LOCAL, Q * O, 2)
        outs.append((v[..., 0] + 1j * v[..., 1]).astype(np.complex64))
    return np.concatenate(outs, axis=0)


# revision 15
# speedup vs baseline: 1.0948x; 1.0948x over previous
"""TRN2 Bass kernel for nn_CustomLinear_66005057405513.

Computes y = FFT_4096(w * x)[:, :3072] for x: [4096, 4096] complex64
(given as interleaved float pairs) and w: [4096] complex64 twiddles.

Strategy: data-parallel over 8 NeuronCores (512 batch rows each). On each
core, a two-stage radix-64 FFT with all twiddles folded into precomputed
matrices:

  n = 64*o + i, k = p + 64*q, q < 48:
    stage 1 (per i):  A[b, i, p] = sum_o C1[i][o, p] * x[b, 64o+i]
                      C1[i][o, p] = W64^(op) * w[64o+i]
    stage 2 (per p):  y[b, p+64q] = sum_i C2[p][i, q] * A[b, i, p]
                      C2[p][i, q] = W4096^(ip) * W64^(iq)

Complex values ride as interleaved (re, im) float pairs; each complex
matmul is one real matmul with the pair-encoded matrix (contraction
K = 128 = full PE partition dim), fp16 compute with fp32 PSUM accum.

The kernel is PE-sequencer-bound: every matmul/transpose costs ~100ns
of issue (Ldweights + dispatch) regardless of size, so the design
minimizes PE instruction count (576 pairs/iteration):

- x is pre-transposed on the host to [oc, (slab, i, b)]: stage 1 needs
  no transposes (256 pairs, N=128 forced by the per-i tables).
- The structurally-required corner-turn of A runs as 256 PE transposes.
- Stage 2 is 64 pairs: one N=512 matmul per p (stationary = the
  constant C2[p] table, moving = all 512 batch columns of transposed A).
- Stage-2 p-range g follows transpose group g of the last slab, so the
  tail interleaves and PSUM never serializes the pipeline.
- Output is y^T [qc, (p, s, b)]; the host untangles it for free.
"""

import numpy as np

import concourse.bass as bass
import concourse.mybir as mybir
from concourse import bacc
from concourse.tile import TileContext
from concourse.masks import make_identity
from concourse.bass_utils import run_bass_kernel_spmd

O = I = 64
N_FFT = O * I          # 4096
Q = 48                 # q < 48  <=>  k < 3072
QC = 2 * Q             # 96
B_TOTAL = 4096
N_CORES = 8
B_LOCAL = B_TOTAL // N_CORES  # 512
SLABS = B_LOCAL // 128        # 4
YCOLS = O * B_LOCAL           # 32768


def _make_tables(w_complex):
    oo = np.arange(O)
    W64 = np.exp(-2j * np.pi * np.outer(oo, oo) / O)
    WN = np.exp(-2j * np.pi * np.outer(np.arange(I), oo) / N_FFT)

    def pairmat(C):
        K, M = C.shape
        G = np.empty((2 * K, 2 * M), np.float64)
        G[0::2, 0::2] = C.real
        G[1::2, 0::2] = -C.imag
        G[0::2, 1::2] = C.imag
        G[1::2, 1::2] = C.real
        return G

    g1 = np.empty((128, I, 128), np.float64)
    for i in range(I):
        C1 = W64 * w_complex[64 * oo + i][:, None]
        g1[:, i, :] = pairmat(C1)
    g2 = np.empty((128, O, QC), np.float64)
    for p in range(O):
        C2 = WN[:, p][:, None] * W64[:, :Q]
        g2[:, p, :] = pairmat(C2)
    return g1, g2


def _build_nc(reps=1, unroll=False):
    f32 = mybir.dt.float32
    f16 = mybir.dt.float16

    nc = bacc.Bacc(None, target_bir_lowering=False, debug=False)
    # x host layout: [oc, (slab, i, b)] -- already transposed for stage 1
    x = nc.declare_dram_parameter("x", [128, SLABS * 8192], f16, isOutput=False)
    w1 = nc.declare_dram_parameter("w1", [128, I * 128], f16, isOutput=False)
    w2 = nc.declare_dram_parameter("w2", [128, O * QC], f16, isOutput=False)
    # y device layout: [qc, (p, s, b)] -- host untangles to [b, k]
    y = nc.declare_dram_parameter("y", [QC, YCOLS], f16, isOutput=True)

    yc = [0]

    def copy_y(out_ap, in_ap):
        # ~25 of every 64 y-evacs on ACT, rest on DVE (measured balance)
        k = yc[0] % 8
        yc[0] += 1
        if k in (0, 3, 5):
            nc.scalar.copy(out_ap, in_ap)
        else:
            nc.vector.tensor_copy(out_ap, in_ap)

    with TileContext(nc) as tc:
        with (
            tc.tile_pool(name="const", bufs=1) as cpool,
            tc.tile_pool(name="xp", bufs=2) as xpool,
            tc.tile_pool(name="ap", bufs=2) as apool,
            tc.tile_pool(name="at", bufs=1) as atpool,
            tc.tile_pool(name="yp", bufs=2) as ypool,
            tc.tile_pool(name="pm1", bufs=2, space="PSUM") as pm1,
            tc.tile_pool(name="pt2", bufs=2, space="PSUM") as pt2,
            tc.tile_pool(name="pm2", bufs=2, space="PSUM") as pm2,
        ):
            ident = cpool.tile([128, 128], f16, name="ident")
            make_identity(nc, ident[:])
            w1s = cpool.tile([128, I * 128], f16, name="w1s")
            nc.scalar.dma_start(out=w1s[:], in_=w1[:])
            w2s = cpool.tile([128, O * QC], f16, name="w2s")
            nc.scalar.dma_start(out=w2s[:], in_=w2[:])
            w1v = w1s[:].rearrange("k (i n) -> k i n", i=I)
            w2v = w2s[:].rearrange("k (p n) -> k p n", p=O)

            def job(_iv=None):
                # At: [ic, (p, s, b)] -- contiguous per-p reads for stage 2
                At = atpool.tile([128, SLABS * 8192], f16, name="At")
                atv = At[:].rearrange("k (p s b) -> k p s b", s=SLABS, p=O)
                live = {}

                def s1_group(g):
                    xv, av = live["xv"], live["av"]
                    m1 = pm1.tile([128, 1024], f32, name="m1")
                    for j in range(8):
                        i = g * 8 + j
                        nc.tensor.matmul(
                            m1[:, j * 128:(j + 1) * 128],
                            lhsT=xv[:, i, :], rhs=w1v[:, i, :],
                            start=True, stop=True)
                    nc.scalar.copy(
                        av[:, g * 8:(g + 1) * 8, :, :],
                        m1[:].rearrange("b (i p c) -> b i p c", i=8, c=2))

                def tr_group(s, g):
                    Ab = live["Ab%d" % s]
                    t2p = pt2.tile([128, 1024], f16, name="t2p")
                    for j in range(8):
                        p = g * 8 + j
                        nc.tensor.transpose(
                            t2p[:, j * 128:(j + 1) * 128],
                            Ab[:, p * 128:(p + 1) * 128], ident[:])
                    nc.vector.tensor_copy(
                        atv[:, g * 8:(g + 1) * 8, s, :],
                        t2p[:].rearrange("k (p b) -> k p b", p=8))

                def s2_one(p):
                    yst = live["yst"]
                    m2 = pm2.tile([QC, 512], f32, name="m2")
                    nc.tensor.matmul(
                        m2[:], lhsT=w2v[:, p, :],
                        rhs=At[:, p * B_LOCAL:(p + 1) * B_LOCAL],
                        start=True, stop=True)
                    copy_y(yst[:, (p % 16) * 512:(p % 16 + 1) * 512], m2[:])

                for s in range(SLABS):
                    xs = xpool.tile([128, 8192], f16, name="xs")
                    for hh in range(2):
                        nc.sync.dma_start(
                            out=xs[:, hh * 4096:(hh + 1) * 4096],
                            in_=x[:, s * 8192 + hh * 4096:
                                  s * 8192 + (hh + 1) * 4096])
                    live["xv"] = xs[:].rearrange("k (i b) -> k i b", i=I)
                    Ab = apool.tile([128, 8192], f16, name="Ab")
                    live["Ab%d" % s] = Ab
                    live["av"] = Ab[:].rearrange("b (p i c) -> b i p c", p=O, c=2)
                    for g in range(8):
                        s1_group(g)
                        if s > 0:
                            tr_group(s - 1, g)
                # tail: tr(3, g) immediately followed by s2 of that p-range
                for g in range(8):
                    tr_group(SLABS - 1, g)
                    if g % 2 == 0:
                        live["yst"] = ypool.tile([QC, 8192], f16, name="yst")
                    for p in range(8 * g, 8 * g + 8):
                        s2_one(p)
                    if g % 2 == 1:
                        nc.scalar.dma_start(
                            out=y[:, (g - 1) * 4096:(g + 1) * 4096],
                            in_=live["yst"][:])

            if reps > 1 and unroll:
                for _ in range(reps):
                    job()
            elif reps > 1:
                with tc.For_i(0, reps, 1) as _i:
                    job(_i)
            else:
                job()

    nc.compile()
    return nc


_NC_CACHE = {}


def _get_nc():
    if "nc" not in _NC_CACHE:
        _NC_CACHE["nc"] = _build_nc()
    return _NC_CACHE["nc"]


def _host_inputs(x_real, weights_real):
    wr = np.asarray(weights_real, dtype=np.float64)
    wc = wr[0::2] + 1j * wr[1::2]
    g1, g2 = _make_tables(wc)
    w1 = np.ascontiguousarray(g1.reshape(128, -1)).astype(np.float16)
    w2 = np.ascontiguousarray(g2.reshape(128, -1)).astype(np.float16)
    x = np.asarray(x_real)
    B = x.shape[0]
    bl = B // N_CORES
    xh = x.reshape(B, O, I, 2).astype(np.float16)  # [b, o, i, c]
    maps = []
    for c in range(N_CORES):
        xc = xh[c * bl:(c + 1) * bl].reshape(SLABS, 128, O, I, 2)
        # [bt, b, o, i, c] -> [o, c, bt, i, b] = [oc, (slab, i, b)]
        xf = np.ascontiguousarray(xc.transpose(2, 4, 0, 3, 1)).reshape(128, -1)
        maps.append({"x": xf, "w1": w1, "w2": w2})
    return maps


def kernel(x_real, weights_real):
    nc = _get_nc()
    in_maps = _host_inputs(x_real, weights_real)
    res = run_bass_kernel_spmd(nc, in_maps, list(range(N_CORES)))
    outs = []
    for c in range(N_CORES):
        v = np.asarray(res.results[c]["y"], dtype=np.float32)
        # v[2q+c, (p, s, b)] -> Y[s*128+b, q*64+p]
        v = v.reshape(Q, 2, O, SLABS, 128).transpose(3, 4, 0, 2, 1)
        v = np.ascontiguousarray(v).reshape(B_LOCAL, Q * O, 2)
        outs.append((v[..., 0] + 1j * v[..., 1]).astype(np.complex64))
    return np.concatenate(outs, axis=0)


# revision 23
# speedup vs baseline: 1.2978x; 1.1855x over previous
"""TRN2 Bass kernel for nn_CustomLinear_66005057405513.

Computes y = FFT_4096(w * x)[:, :3072] for x: [4096, 4096] complex64
(given as interleaved float pairs) and w: [4096] complex64 twiddles.

Strategy: data-parallel over 8 NeuronCores (512 batch rows each). On each
core, a two-stage radix-64 FFT with all twiddles folded into precomputed
matrices:

  n = 64*o + i, k = p + 64*q, q < 48:
    stage 1 (per i):  A[b, i, p] = sum_o C1[i][o, p] * x[b, 64o+i]
                      C1[i][o, p] = W64^(op) * w[64o+i]
    stage 2 (per p):  y[b, p+64q] = sum_i C2[p][i, q] * A[b, i, p]
                      C2[p][i, q] = W4096^(ip) * W64^(iq)

Complex values ride as interleaved (re, im) float pairs; each complex
matmul is one real matmul with the pair-encoded matrix (contraction
K = 128 = full PE partition dim), fp16 compute with fp32 PSUM accum.

The kernel is PE-sequencer-bound: every matmul/transpose costs ~100ns
of issue (Ldweights + dispatch) regardless of size, so the design
minimizes PE instruction count (576 pairs/iteration):

- x is pre-transposed on the host to [oc, (slab, i, b)]: stage 1 needs
  no transposes (256 pairs, N=128 forced by the per-i tables).
- The structurally-required corner-turn of A runs as 256 PE transposes.
- Stage 2 is 64 pairs: one N=512 matmul per p (stationary = the
  constant C2[p] table, moving = all 512 batch columns of transposed A).
- Stage-2 p-range g follows transpose group g of the last slab, so the
  tail interleaves and PSUM never serializes the pipeline.
- Output is y^T [qc, (p, s, b)]; the host untangles it for free.
"""

import numpy as np

import concourse.bass as bass
import concourse.mybir as mybir
from concourse import bacc
from concourse.tile import TileContext
from concourse.masks import make_identity
from concourse.bass_utils import run_bass_kernel_spmd

O = I = 64
N_FFT = O * I          # 4096
Q = 48                 # q < 48  <=>  k < 3072
QC = 2 * Q             # 96
B_TOTAL = 4096
N_CORES = 8
B_LOCAL = B_TOTAL // N_CORES  # 512
SLABS = B_LOCAL // 128        # 4
YCOLS = O * B_LOCAL           # 32768


def _make_tables(w_complex):
    oo = np.arange(O)
    W64 = np.exp(-2j * np.pi * np.outer(oo, oo) / O)
    WN = np.exp(-2j * np.pi * np.outer(np.arange(I), oo) / N_FFT)

    def pairmat(C):
        K, M = C.shape
        G = np.empty((2 * K, 2 * M), np.float64)
        G[0::2, 0::2] = C.real
        G[1::2, 0::2] = -C.imag
        G[0::2, 1::2] = C.imag
        G[1::2, 1::2] = C.real
        return G

    g1 = np.empty((128, I, 128), np.float64)
    for i in range(I):
        C1 = W64 * w_complex[64 * oo + i][:, None]
        g1[:, i, :] = pairmat(C1)
    g2 = np.empty((128, O, QC), np.float64)
    for p in range(O):
        C2 = WN[:, p][:, None] * W64[:, :Q]
        g2[:, p, :] = pairmat(C2)
    return g1, g2


def _build_nc(reps=1, unroll=False, per_iter=8):
    f32 = mybir.dt.float32
    f16 = mybir.dt.float16

    nc = bacc.Bacc(None, target_bir_lowering=False, debug=False)
    # x host layout: [oc, (slab, i, b)] -- already transposed for stage 1
    x = nc.declare_dram_parameter("x", [128, SLABS * 8192], f16, isOutput=False)
    w1 = nc.declare_dram_parameter("w1", [128, I * 128], f16, isOutput=False)
    w2 = nc.declare_dram_parameter("w2", [128, O * QC], f16, isOutput=False)
    # y device layout: [qc, (p, s, b)] -- host untangles to [b, k]
    y = nc.declare_dram_parameter("y", [QC, YCOLS], f16, isOutput=True)

    yc = [0]

    def copy_y(out_ap, in_ap):
        # ~25 of every 64 y-evacs on ACT, rest on DVE (measured balance)
        k = yc[0] % 8
        yc[0] += 1
        if k in (0, 3, 5):
            nc.scalar.copy(out_ap, in_ap)
        else:
            nc.vector.tensor_copy(out_ap, in_ap)

    with TileContext(nc) as tc:
        with (
            tc.tile_pool(name="const", bufs=1) as cpool,
            tc.tile_pool(name="xp", bufs=2) as xpool,
            tc.tile_pool(name="ap", bufs=2) as apool,
            tc.tile_pool(name="at", bufs=1) as atpool,
            tc.tile_pool(name="yp", bufs=2) as ypool,
            tc.tile_pool(name="pm1", bufs=2, space="PSUM") as pm1,
            tc.tile_pool(name="pt2", bufs=2, space="PSUM") as pt2,
            tc.tile_pool(name="pm2", bufs=2, space="PSUM") as pm2,
        ):
            ident = cpool.tile([128, 128], f16, name="ident")
            make_identity(nc, ident[:])
            w1s = cpool.tile([128, I * 128], f16, name="w1s")
            nc.scalar.dma_start(out=w1s[:], in_=w1[:])
            w2s = cpool.tile([128, O * QC], f16, name="w2s")
            nc.scalar.dma_start(out=w2s[:], in_=w2[:])
            w1v = w1s[:].rearrange("k (i n) -> k i n", i=I)
            w2v = w2s[:].rearrange("k (p n) -> k p n", p=O)

            def job(_iv=None):
                # At: [ic, (p, s, b)] -- contiguous per-p reads for stage 2
                At = atpool.tile([128, SLABS * 8192], f16, name="At")
                atv = At[:].rearrange("k (p s b) -> k p s b", s=SLABS, p=O)
                live = {}

                def s1_group(g):
                    xv, av = live["xv"], live["av"]
                    m1 = pm1.tile([128, 1024], f32, name="m1")
                    for j in range(8):
                        i = g * 8 + j
                        nc.tensor.matmul(
                            m1[:, j * 128:(j + 1) * 128],
                            lhsT=xv[:, i, :], rhs=w1v[:, i, :],
                            start=True, stop=True)
                    nc.scalar.copy(
                        av[:, g * 8:(g + 1) * 8, :, :],
                        m1[:].rearrange("b (i p c) -> b i p c", i=8, c=2))

                def tr_group(s, g):
                    Ab = live["Ab%d" % s]
                    t2p = pt2.tile([128, 1024], f16, name="t2p")
                    for j in range(8):
                        p = g * 8 + j
                        nc.tensor.transpose(
                            t2p[:, j * 128:(j + 1) * 128],
                            Ab[:, p * 128:(p + 1) * 128], ident[:])
                    nc.vector.tensor_copy(
                        atv[:, g * 8:(g + 1) * 8, s, :],
                        t2p[:].rearrange("k (p b) -> k p b", p=8))

                def s2_one(p):
                    yst = live["yst"]
                    m2 = pm2.tile([QC, 512], f32, name="m2")
                    nc.tensor.matmul(
                        m2[:], lhsT=w2v[:, p, :],
                        rhs=At[:, p * B_LOCAL:(p + 1) * B_LOCAL],
                        start=True, stop=True)
                    copy_y(yst[:, (p % 16) * 512:(p % 16 + 1) * 512], m2[:])

                for s in range(SLABS):
                    xs = xpool.tile([128, 8192], f16, name="xs")
                    for hh in range(2):
                        nc.sync.dma_start(
                            out=xs[:, hh * 4096:(hh + 1) * 4096],
                            in_=x[:, s * 8192 + hh * 4096:
                                  s * 8192 + (hh + 1) * 4096])
                    live["xv"] = xs[:].rearrange("k (i b) -> k i b", i=I)
                    Ab = apool.tile([128, 8192], f16, name="Ab")
                    live["Ab%d" % s] = Ab
                    live["av"] = Ab[:].rearrange("b (p i c) -> b i p c", p=O, c=2)
                    for g in range(8):
                        s1_group(g)
                        if s > 0:
                            tr_group(s - 1, g)
                # tail: tr(3, g) immediately followed by s2 of that p-range
                for g in range(8):
                    tr_group(SLABS - 1, g)
                    if g % 2 == 0:
                        live["yst"] = ypool.tile([QC, 8192], f16, name="yst")
                    for p in range(8 * g, 8 * g + 8):
                        s2_one(p)
                    if g % 2 == 1:
                        nc.scalar.dma_start(
                            out=y[:, (g - 1) * 4096:(g + 1) * 4096],
                            in_=live["yst"][:])

            if reps > 1 and unroll:
                for _ in range(reps):
                    job()
            elif reps > 1:
                # unroll jobs inside the hardware loop: the For_i iteration
                # boundary is a partial barrier (~13us/job measured), so
                # amortize it over per_iter jobs; remainder runs after.
                n_unroll = min(per_iter, reps)
                with tc.For_i(0, reps // n_unroll, 1) as _i:
                    for _ in range(n_unroll):
                        job(_i)
                for _ in range(reps % n_unroll):
                    job()
            else:
                job()

    nc.compile()
    return nc


_NC_CACHE = {}


def _get_nc():
    if "nc" not in _NC_CACHE:
        _NC_CACHE["nc"] = _build_nc()
    return _NC_CACHE["nc"]


def _host_inputs(x_real, weights_real):
    wr = np.asarray(weights_real, dtype=np.float64)
    wc = wr[0::2] + 1j * wr[1::2]
    g1, g2 = _make_tables(wc)
    w1 = np.ascontiguousarray(g1.reshape(128, -1)).astype(np.float16)
    w2 = np.ascontiguousarray(g2.reshape(128, -1)).astype(np.float16)
    x = np.asarray(x_real)
    B = x.shape[0]
    bl = B // N_CORES
    xh = x.reshape(B, O, I, 2).astype(np.float16)  # [b, o, i, c]
    maps = []
    for c in range(N_CORES):
        xc = xh[c * bl:(c + 1) * bl].reshape(SLABS, 128, O, I, 2)
        # [bt, b, o, i, c] -> [o, c, bt, i, b] = [oc, (slab, i, b)]
        xf = np.ascontiguousarray(xc.transpose(2, 4, 0, 3, 1)).reshape(128, -1)
        maps.append({"x": xf, "w1": w1, "w2": w2})
    return maps


def kernel(x_real, weights_real):
    nc = _get_nc()
    in_maps = _host_inputs(x_real, weights_real)
    res = run_bass_kernel_spmd(nc, in_maps, list(range(N_CORES)))
    outs = []
    for c in range(N_CORES):
        v = np.asarray(res.results[c]["y"], dtype=np.float32)
        # v[2q+c, (p, s, b)] -> Y[s*128+b, q*64+p]
        v = v.reshape(Q, 2, O, SLABS, 128).transpose(3, 4, 0, 2, 1)
        v = np.ascontiguousarray(v).reshape(B_LOCAL, Q * O, 2)
        outs.append((v[..., 0] + 1j * v[..., 1]).astype(np.complex64))
    return np.concatenate(outs, axis=0)
